# Initial kernel scaffold
#
"""Trainium2 Bass kernel for YOLO-style detection decode (nms_detection).

Computes, for input `output` (B=8, H=80, W=80, A*85=255):
  per (b, cell, anchor):  xy = (sigmoid(txy) + grid_off) * stride
                          wh = exp(twh) * anchor
                          bbox = [xy - wh/2, xy + wh/2]
                          p_c = sigmoid(cls_c) * sigmoid(obj)
  out (B, C*hw*A, 6) rows = [cid, score, x1, y1, x2, y2] where
  cid = c if p_c > 0.01 else -1, score = p_c if p_c > 0.01 else 0.

Sharding: pure data parallel over batch, one batch element per NeuronCore.

Per-core layout strategy (output is 37 MB/core -> write-bandwidth bound):
  - cells are processed in supertiles of 512 (4 subtiles of 128 = partition dim)
  - class scores are transposed (TensorE) to class-major (80 partitions) so the
    per-class output block (hw, A, 6) is DMA'd with 9 KB contiguous segments
  - bbox (class-independent) is broadcast to all 80 class partitions with
    one-hot "selector" matmuls (K=12) on the otherwise idle TensorE
  - exp(x) is computed as sigmoid(x)/sigmoid(-x) so the ScalarE activation
    table never leaves the sigmoid set (a table switch costs ~2.7us).
"""

import sys
import os
from contextlib import ExitStack

if "/opt/trn_rl_repo" not in sys.path:
    sys.path.insert(0, "/opt/trn_rl_repo")

import numpy as np

NUM_CLASSES = 80
NUM_ANCHOR = 3
NUM_PRED = 85
HW_CELLS = 6400
THRESH = 0.01
N_CORES = 8
ROW = 6 * NUM_ANCHOR  # f32 per cell per class in the output (18)

_CACHE = {}
LAST_RESULT = None  # BassKernelResults of the most recent kernel() call


# Row groups used for the broadcast matmuls (1, 2, or 4). Multi-group runs
# the per-anchor broadcasts concurrently on distinct PE row groups.
ROW_GROUPS = int(os.environ.get("KERNEL_ROW_GROUPS", "1"))
BASES = (96, 0, 32, 64)
BCAST_DTYPE = os.environ.get("KERNEL_BCAST_DTYPE", "f32r")  # f32r | f32


def _build(stride_f: float, n_cells: int = HW_CELLS):
    import concourse.bass as bass  # noqa: F401
    import concourse.bacc as bacc
    import concourse.tile as tile
    from concourse import mybir

    f32 = mybir.dt.float32
    f32r = mybir.dt.float32r
    AF = mybir.ActivationFunctionType
    OP = mybir.AluOpType

    C = NUM_CLASSES
    A = NUM_ANCHOR

    n_tiles = n_cells // 128
    CONST_F = n_tiles * 6 + 24 + 128 + 1 + 12 * C  # offs | hanch | ident | cp1 | sel
    OFF_HANCH = n_tiles * 6
    OFF_IDENT = OFF_HANCH + 24
    OFF_CP1 = OFF_IDENT + 128
    OFF_SEL = OFF_CP1 + 1

    nc = bacc.Bacc("TRN2", target_bir_lowering=False, debug=False)
    x_d = nc.declare_dram_parameter("x", [n_cells, A * NUM_PRED], f32, isOutput=False)
    const_d = nc.declare_dram_parameter("consts", [128, CONST_F], f32, isOutput=False)
    out_d = nc.declare_dram_parameter("out", [C, n_cells * ROW], f32, isOutput=True)

    # supertile = up to 4 subtiles of 128 cells
    st_sizes = []
    left = n_cells
    while left > 0:
        take = min(512, left)
        assert take % 128 == 0
        st_sizes.append(take)
        left -= take

    with ExitStack() as ctx:
        tc = ctx.enter_context(tile.TileContext(nc))
        cpool = ctx.enter_context(tc.tile_pool(name="const", bufs=1))
        in_pool = ctx.enter_context(tc.tile_pool(name="inp", bufs=3))
        sig_pool = ctx.enter_context(tc.tile_pool(name="sig", bufs=2))
        sm_pool = ctx.enter_context(tc.tile_pool(name="small", bufs=2))
        s_pool = ctx.enter_context(tc.tile_pool(name="scls", bufs=2))
        m_pool = ctx.enter_context(tc.tile_pool(name="mask", bufs=3))
        stg_pool = ctx.enter_context(tc.tile_pool(name="stage", bufs=2))
        o_pool = ctx.enter_context(tc.tile_pool(name="outt", bufs=2))
        p_pool = ctx.enter_context(tc.tile_pool(name="ppsum", bufs=2, space="PSUM"))
        q_pool = ctx.enter_context(tc.tile_pool(name="qpsum", bufs=3, space="PSUM"))

        # ---- constants (one DMA -> one sem lane) ----
        const_sb = cpool.tile([128, CONST_F], f32, tag="consts")
        nc.scalar.dma_start(out=const_sb[:, :], in_=const_d[:, :])
        offs_sb = const_sb[:, 0:OFF_HANCH]
        hanch_sb = const_sb[:, OFF_HANCH:OFF_IDENT]
        ident_sb = const_sb[:, OFF_IDENT:OFF_CP1]
        cp1_sb = const_sb[:, OFF_CP1:OFF_SEL]
        sel_sb = const_sb[:, OFF_SEL:CONST_F]
        bc_dt = f32r if BCAST_DTYPE == "f32r" else f32
        sel_r = cpool.tile([128, 12 * C], bc_dt, tag="selr")
        nc.vector.tensor_copy(sel_r[:, :], sel_sb)

        # ---- warm-up: let each engine observe the const DMA once, so no
        # later instruction needs more than one sync-wait (ISA limit) ----
        warm = cpool.tile([128, 4], f32, tag="warm")
        nc.vector.tensor_copy(warm[0:1, 0:1], const_sb[0:1, 0:1])
        nc.scalar.copy(warm[0:1, 1:2], const_sb[0:1, 0:1])
        nc.gpsimd.tensor_copy(warm[0:1, 2:3], const_sb[0:1, 0:1])
        wq = p_pool.tile([128, 128], f32, tag="P")
        nc.tensor.transpose(wq[:, :], ident_sb, ident_sb)

        c0 = 0
        for st, ncell in enumerate(st_sizes):
            ns = ncell // 128  # subtiles
            t0 = c0 // 128

            # ---- load input supertile: [p, s, 255] ----
            in_t = in_pool.tile([128, ns * 255], f32, tag="in")
            nc.scalar.dma_start(
                out=in_t[:, :].rearrange("p (s c) -> p s c", c=255),
                in_=x_d[c0 : c0 + ncell, :].rearrange("(s p) c -> p s c", p=128),
            )

            # ---- cell-major transforms ----
            sig = sig_pool.tile([128, ns * 255], f32, tag="sig")
            nc.scalar.activation(sig[:, :], in_t[:, :], AF.Sigmoid)

            in_v = in_t[:, :].rearrange("p (s a c) -> p s a c", a=A, c=NUM_PRED)
            sig_v = sig[:, :].rearrange("p (s a c) -> p s a c", a=A, c=NUM_PRED)

            # exp(wh) = sigmoid(wh) / sigmoid(-wh)
            sgnw = sm_pool.tile([128, ns * 6], f32, tag="sgnw")
            nc.scalar.activation(
                sgnw[:, :].rearrange("p (s a k) -> p s a k", a=A, k=2),
                in_v[:, :, :, 2:4],
                AF.Sigmoid,
                scale=-1.0,
            )
            rec = sm_pool.tile([128, ns * 6], f32, tag="rec")
            nc.vector.reciprocal(rec[:, :], sgnw[:, :])
            t1 = sm_pool.tile([128, ns * 6], f32, tag="t1")
            nc.vector.tensor_tensor(
                t1[:, :].rearrange("p (s a k) -> p s a k", a=A, k=2),
                sig_v[:, :, :, 2:4],
                hanch_sb[:, : ns * 6].rearrange("p (s a k) -> p s a k", a=A, k=2),
                OP.mult,
            )
            halfwh = sm_pool.tile([128, ns * 6], f32, tag="halfwh")
            nc.vector.tensor_tensor(halfwh[:, :], t1[:, :], rec[:, :], OP.mult)

            # xy = sigmoid(xy)*stride + off*stride
            xy = sm_pool.tile([128, ns * 6], f32, tag="xy")
            nc.vector.scalar_tensor_tensor(
                xy[:, :].rearrange("p (s a k) -> p s a k", a=A, k=2),
                in0=sig_v[:, :, :, 0:2],
                scalar=stride_f,
                in1=offs_sb[:, t0 * 6 : (t0 + ns) * 6].rearrange(
                    "p (s a k) -> p s a k", a=A, k=2
                ),
                op0=OP.mult,
                op1=OP.add,
            )

            # per-subtile block layout [S_a0 | S_a1 | S_a2 | pad 16 | bb 12] so
            # anchor 2's transpose carries the bbox columns for free, landing
            # them on PE row group 3 (partitions 96..107)
            SW = A * C + 16 + 12  # 268
            S = s_pool.tile([128, ns * SW], f32, tag="S")

            # bbox cell-major -> S cols [240:252) per subtile: [a, 4] = x1 y1 x2 y2
            S_v = S[:, :].rearrange("p (s w) -> p s w", w=SW)
            bb_v = S[:, :].rearrange("p (s w) -> p s w", w=SW)[
                :, :, A * C + 16 : SW
            ].rearrange("p s (a k) -> p s a k", k=4)
            xy_v = xy[:, :].rearrange("p (s a k) -> p s a k", a=A, k=2)
            hw_v = halfwh[:, :].rearrange("p (s a k) -> p s a k", a=A, k=2)
            nc.vector.tensor_tensor(bb_v[:, :, :, 0:2], xy_v, hw_v, OP.subtract)
            nc.vector.tensor_tensor(bb_v[:, :, :, 2:4], xy_v, hw_v, OP.add)

            # class scores = sigmoid(cls) * sigmoid(obj), cell-major; obj is
            # broadcast along the class dim with a stride-0 AP (gpsimd: DVE
            # and ACT are the busy engines)
            nc.gpsimd.tensor_tensor(
                S[:, :]
                .rearrange("p (s w) -> p s w", w=SW)[:, :, 0 : A * C]
                .rearrange("p s (a c) -> p s a c", c=C),
                sig_v[:, :, :, 5:85],
                sig_v[:, :, :, 4:5].to_broadcast([128, ns, A, C]),
                OP.mult,
            )

            # output supertile, class-major
            outt = o_pool.tile([C, ncell * ROW], f32, tag="outt")
            ov = outt[:, :].rearrange("c (i e) -> c e i", e=ROW)

            bbt = stg_pool.tile([128, ncell], bc_dt, tag="bbt")

            for a in (2, 0, 1):  # anchor 2 first: it stages the bbox rows
                # transpose scores of anchor a -> [C, ncell]; anchor 2 also
                # carries the 12 bbox rows into partitions 96..107
                pw = C + 28 if a == 2 else C
                P = p_pool.tile([C + 28, ncell], f32, tag="P")
                for s in range(ns):
                    nc.tensor.transpose(
                        P[0:pw, s * 128 : (s + 1) * 128],
                        S_v[:, s, a * C : a * C + pw],
                        ident_sb[:, :],
                    )
                if a == 2:
                    # stage bbox rows to SBUF (rounds to fp32r) on row group 3
                    nc.vector.tensor_copy(bbt[96:108, :], P[96:108, :])
                    for base in BASES[1:ROW_GROUPS]:
                        nc.sync.dma_start(
                            out=bbt[base : base + 12, :], in_=bbt[96:108, :]
                        )
                mask = m_pool.tile([C, ncell], f32, tag="mask")
                nc.vector.tensor_scalar(mask[:, :], P[0:C, :], THRESH, None, OP.is_gt)
                # score -> column a*6+1 (strided 18)
                nc.vector.tensor_tensor(
                    ov[:, a * 6 + 1, :], P[0:C, :], mask[:, :], OP.mult
                )
                # cid = mask*(c+1) - 1 -> column a*6+0 (gpsimd, SBUF-only op)
                nc.gpsimd.tensor_scalar(
                    ov[:, a * 6 + 0, :],
                    mask[:, :],
                    cp1_sb[0:C, :],
                    -1.0,
                    OP.mult,
                    OP.add,
                )

                # bbox broadcast via one-hot selector matmuls (fp32r, single
                # pass, always row group 3)
                for half in range(2):
                    q = q_pool.tile([C, 2 * ncell], f32, tag="q")
                    for kk in range(2):
                        k = half * 2 + kk
                        j = a * 4 + k
                        base = BASES[j % ROW_GROUPS]
                        nc.tensor.matmul(
                            q[:, kk * ncell : (kk + 1) * ncell],
                            lhsT=sel_r[base : base + 12, j * C : (j + 1) * C],
                            rhs=bbt[base : base + 12, :],
                            start=True,
                            stop=True,
                            tile_position=(base, 0),
                        )
                    dst = ov[:, a * 6 + 2 + half * 2 : a * 6 + 4 + half * 2, :]
                    src = q[:, :].rearrange("c (k i) -> c k i", k=2)
                    if (a, half) in ((0, 1), (1, 1)):
                        nc.vector.tensor_copy(dst, src)
                    else:
                        nc.scalar.copy(dst, src)

            # ---- store ----
            nc.sync.dma_start(
                out=out_d[:, c0 * ROW : (c0 + ncell) * ROW], in_=outt[:, :]
            )
            c0 += ncell

    nc.finalize()
    return nc


def make_consts(anchor, offset, stride_f, n_cells=HW_CELLS):
    """Pack [offs | hanch | ident | cp1 | sel] into one (128, F) f32 blob."""
    n_tiles = n_cells // 128
    off = np.asarray(offset, dtype=np.float32).reshape(-1, 2)[:n_cells] * stride_f
    offs6 = np.tile(off, (1, 3)).reshape(n_tiles, 128, 6)  # [t, p, j]
    offs_cols = np.ascontiguousarray(np.transpose(offs6, (1, 0, 2)).reshape(128, n_tiles * 6))
    a2 = np.asarray(anchor, dtype=np.float32).reshape(NUM_ANCHOR, 2)
    hanch = np.tile((a2 / 2.0).reshape(6), (128, 4)).astype(np.float32)
    ident = np.eye(128, dtype=np.float32)
    cp1 = np.broadcast_to(np.arange(1, 129, dtype=np.float32).reshape(128, 1), (128, 1))
    # one-hot selector for bbox channel j, placed on PE row group j%4 so the
    # four per-anchor broadcast matmuls can row-tile concurrently
    # one-hot selectors for bbox channel j, for every row-group mapping the
    # kernel might use (distinct bases never collide within a column block)
    sel128 = np.zeros((128, 12 * NUM_CLASSES), dtype=np.float32)
    bases = (96, 0, 32, 64)
    for rg in (1, 2, 4):
        for j in range(12):
            sel128[bases[j % rg] + j, j * NUM_CLASSES : (j + 1) * NUM_CLASSES] = 1.0
    blob = np.concatenate([offs_cols, hanch, ident, cp1, sel128], axis=1)
    return np.ascontiguousarray(blob.astype(np.float32))


def _host_prep(output, anchor, offset, stride):
    stride_f = float(stride)
    B = output.shape[0]
    x_all = np.ascontiguousarray(
        np.asarray(output, dtype=np.float32).reshape(B, HW_CELLS, NUM_ANCHOR * NUM_PRED)
    )
    consts = make_consts(anchor, offset, stride_f)
    return stride_f, x_all, consts


def kernel(output, anchor, offset, stride):
    from concourse.bass_utils import run_bass_kernel_spmd

    stride_f, x_all, consts = _host_prep(output, anchor, offset, stride)
    key = ("nc", stride_f)
    if key not in _CACHE:
        _CACHE[key] = _build(stride_f)
    nc = _CACHE[key]

    in_maps = [{"x": x_all[b], "consts": consts} for b in range(N_CORES)]
    res = run_bass_kernel_spmd(
        nc,
        in_maps,
        list(range(N_CORES)),
        tmpdir=os.environ.get("KERNEL_TRACE_DIR") or None,
    )
    global LAST_RESULT
    LAST_RESULT = res
    outs = [
        r["out"].reshape(NUM_CLASSES * HW_CELLS * NUM_ANCHOR, 6) for r in res.results
    ]
    return np.stack(outs, axis=0)


if __name__ == "__main__":
    rng = np.random.default_rng(0)
    out = rng.standard_normal((8, 80, 80, 255), dtype=np.float32)
    anchor = rng.uniform(10.0, 120.0, (1, 1, 3, 2)).astype(np.float32)
    gy, gx = np.meshgrid(np.arange(80, dtype=np.float32), np.arange(80, dtype=np.float32), indexing="ij")
    offset = np.stack([gx, gy], axis=-1).reshape(1, 80, 80, 1, 2)
    r = kernel(out, anchor, offset, 8)
    print(r.shape, r.dtype)



# revision 8
# speedup vs baseline: 1.1984x; 1.1984x over previous
"""Trainium2 Bass kernel for YOLO-style detection decode (nms_detection).

Computes, for input `output` (B=8, H=80, W=80, A*85=255):
  per (b, cell, anchor):  xy = (sigmoid(txy) + grid_off) * stride
                          wh = exp(twh) * anchor
                          bbox = [xy - wh/2, xy + wh/2]
                          p_c = sigmoid(cls_c) * sigmoid(obj)
  out (B, C*hw*A, 6) rows = [cid, score, x1, y1, x2, y2] where
  cid = c if p_c > 0.01 else -1, score = p_c if p_c > 0.01 else 0.

Sharding: pure data parallel over batch, one batch element per NeuronCore.

Per-core design (output is 37 MB/core -> store-bandwidth bound):
  - fully CELL-MAJOR pipeline: partition p owns cells [10p, 10p+10) of each
    1280-cell supertile (5 supertiles cover hw=6400). No transposes, no PSUM,
    no TensorE at all; every op runs on 128 partitions.
  - the output staging tile is [128, C, 10, A, 6] (class in the FREE dim);
    the store DMA's DRAM-side AP (p, c, k) = c*115200 + st*23040 + p*180 + k
    writes 720B-contiguous runs per (partition, class) - all 16 SDMA engines
    carry equal load (vs 80-partition class-major staging which overloads the
    even engines 2:1).
  - score & cid each use one fused scalar_tensor_tensor:
      score = (S > t) * S;  cid+1 = (S > t) * (c+1), then ACT applies -1.
  - bbox columns are broadcast across classes with free-dim stride-0 APs,
    split between DVE and ACT so both stay under the DMA shadow.
  - exp(x) = sigmoid(x)/sigmoid(-x) so ScalarE never switches tables.
"""

import sys
import os
from contextlib import ExitStack

if "/opt/trn_rl_repo" not in sys.path:
    sys.path.insert(0, "/opt/trn_rl_repo")

import numpy as np

NUM_CLASSES = 80
NUM_ANCHOR = 3
NUM_PRED = 85
HW_CELLS = 6400
THRESH = 0.01
N_CORES = 8
ROW = 6 * NUM_ANCHOR  # f32 per cell per class in the output (18)

ST_CELLS = 1280          # cells per supertile (5 supertiles)
Q = ST_CELLS // 128      # cells per partition per supertile (10)
N_ST = HW_CELLS // ST_CELLS

# class split point for the bbox broadcast: classes [0, BSPLIT) on DVE,
# [BSPLIT, 80) on ScalarE.  DVE runs fp32 single-src copies at 2 elem/cyc
# (2x_2P), ScalarE at ~1 elem/cyc, so DVE gets the smaller class share only
# because it also carries the score/cid work.
BSPLIT = int(os.environ.get("KERNEL_BSPLIT", "36"))

_CACHE = {}
LAST_RESULT = None  # BassKernelResults of the most recent kernel() call


def _build(stride_f: float):
    import concourse.bass as bass  # noqa: F401
    import concourse.bacc as bacc
    import concourse.tile as tile
    from concourse import mybir

    f32 = mybir.dt.float32
    AF = mybir.ActivationFunctionType
    OP = mybir.AluOpType

    C = NUM_CLASSES
    A = NUM_ANCHOR

    # consts blob: [offs (N_ST*Q*A*2) | hanch (A*2) | cpat (C)]
    OFF_HANCH = N_ST * Q * A * 2   # 300
    OFF_CPAT = OFF_HANCH + A * 2   # 306
    CONST_F = OFF_CPAT + C         # 386

    nc = bacc.Bacc("TRN2", target_bir_lowering=False, debug=False)
    x_d = nc.declare_dram_parameter("x", [HW_CELLS, A * NUM_PRED], f32, isOutput=False)
    const_d = nc.declare_dram_parameter("consts", [128, CONST_F], f32, isOutput=False)
    out_d = nc.declare_dram_parameter("out", [C, HW_CELLS * ROW], f32, isOutput=True)

    with ExitStack() as ctx:
        tc = ctx.enter_context(tile.TileContext(nc))
        cpool = ctx.enter_context(tc.tile_pool(name="const", bufs=1))
        in_pool = ctx.enter_context(tc.tile_pool(name="inp", bufs=2))
        sm_pool = ctx.enter_context(tc.tile_pool(name="small", bufs=2))
        s_pool = ctx.enter_context(tc.tile_pool(name="scls", bufs=2))
        sc_pool = ctx.enter_context(tc.tile_pool(name="scid", bufs=1))
        o_pool = ctx.enter_context(tc.tile_pool(name="outt", bufs=2))

        # ---- constants (one DMA -> one sem lane) ----
        const_sb = cpool.tile([128, CONST_F], f32, tag="consts")
        nc.gpsimd.dma_start(out=const_sb[:, :], in_=const_d[:, :])
        offs_sb = const_sb[:, 0:OFF_HANCH]
        hanch_v = const_sb[:, OFF_HANCH:OFF_CPAT].rearrange(
            "p (u a k) -> p u a k", a=A, k=2
        )
        cpat_v = const_sb[:, OFF_CPAT:CONST_F].rearrange("p (c u) -> p c u", u=1)

        # ---- warm-up: let each engine observe the const DMA once, so no
        # later instruction needs more than one sync-wait (ISA limit) ----
        warm = cpool.tile([128, 4], f32, tag="warm")
        nc.vector.tensor_copy(warm[0:1, 0:1], const_sb[0:1, 0:1])
        nc.scalar.copy(warm[0:1, 1:2], const_sb[0:1, 0:1])
        nc.gpsimd.tensor_copy(warm[0:1, 2:3], const_sb[0:1, 0:1])

        for st in range(N_ST):
            c0 = st * ST_CELLS

            # ---- load input supertile: partition p holds cells [Qp, Qp+Q) ----
            in_t = in_pool.tile([128, Q, A * NUM_PRED], f32, tag="in")
            nc.gpsimd.dma_start(
                out=in_t[:, :, :],
                in_=x_d[c0 : c0 + ST_CELLS, :].rearrange("(p q) c -> p q c", p=128),
            )
            in_v = in_t[:, :, :].rearrange("p q (a c) -> p q a c", a=A, c=NUM_PRED)

            # exp(wh) = sigmoid(wh) / sigmoid(-wh); halfwh = exp(wh)*anchor/2
            # (sgnw reads the raw wh, so it runs before the in-place sigmoid;
            # both are ScalarE ops so they stay ordered)
            sgnw = sm_pool.tile([128, Q, A, 2], f32, tag="sgnw")
            nc.scalar.activation(sgnw[:, :, :, :], in_v[:, :, :, 2:4], AF.Sigmoid, scale=-1.0)

            # ---- sigmoid of everything, in place ----
            nc.scalar.activation(in_t[:, :, :], in_t[:, :, :], AF.Sigmoid)
            sig_v = in_v
            rec = sm_pool.tile([128, Q, A, 2], f32, tag="rec")
            nc.vector.reciprocal(rec[:, :, :, :], sgnw[:, :, :, :])
            t1 = sm_pool.tile([128, Q, A, 2], f32, tag="t1")
            nc.vector.tensor_tensor(
                t1[:, :, :, :],
                sig_v[:, :, :, 2:4],
                hanch_v.to_broadcast([128, Q, A, 2]),
                OP.mult,
            )
            halfwh = sm_pool.tile([128, Q, A, 2], f32, tag="halfwh")
            nc.vector.tensor_tensor(halfwh[:, :, :, :], t1[:, :, :, :], rec[:, :, :, :], OP.mult)

            # xy = sigmoid(xy)*stride + off*stride
            xy = sm_pool.tile([128, Q, A, 2], f32, tag="xy")
            nc.vector.scalar_tensor_tensor(
                xy[:, :, :, :],
                in0=sig_v[:, :, :, 0:2],
                scalar=stride_f,
                in1=offs_sb[:, st * Q * A * 2 : (st + 1) * Q * A * 2].rearrange(
                    "p (q a k) -> p q a k", a=A, k=2
                ),
                op0=OP.mult,
                op1=OP.add,
            )

            # bbox cell-major: bb[p, 1, q, a, 0:2]=xy-halfwh, [2:4]=xy+halfwh
            bb = sm_pool.tile([128, 1, Q, A, 4], f32, tag="bb")
            nc.vector.tensor_tensor(bb[:, 0, :, :, 0:2], xy[:, :, :, :], halfwh[:, :, :, :], OP.subtract)
            nc.vector.tensor_tensor(bb[:, 0, :, :, 2:4], xy[:, :, :, :], halfwh[:, :, :, :], OP.add)

            # class scores S[p, q, a, c] = sigmoid(cls) * sigmoid(obj)
            S = s_pool.tile([128, Q, A, C], f32, tag="S")
            nc.gpsimd.tensor_tensor(
                S[:, :, :, :],
                sig_v[:, :, :, 5:85],
                sig_v[:, :, :, 4:5].to_broadcast([128, Q, A, C]),
                OP.mult,
            )
            S_cqa = S[:, :, :, :].rearrange("p q a c -> p c (q a)")

            # output supertile [p, c, q, a, 6]
            ov = o_pool.tile([128, C, Q, A, 6], f32, tag="ov")
            ov_col = ov[:, :, :, :, :].rearrange("p c q a e -> p c (q a) e")

            # score/cid staged in contiguous planes (strided single-column
            # writes run at <=1/2 rate; contiguous stt + 2x-mode copy wins)
            sc_t = sc_pool.tile([128, 2, C, Q * A], f32, tag="sc")
            # plane 1: score = (S > t) * S
            nc.vector.scalar_tensor_tensor(
                sc_t[:, 1, :, :],
                in0=S_cqa,
                scalar=THRESH,
                in1=S_cqa,
                op0=OP.is_gt,
                op1=OP.mult,
            )
            # plane 0: cid+1 = (S > t) * (c+1), then -1 applied contiguously
            nc.vector.scalar_tensor_tensor(
                sc_t[:, 0, :, :],
                in0=S_cqa,
                scalar=THRESH,
                in1=cpat_v.to_broadcast([128, C, Q * A]),
                op0=OP.is_gt,
                op1=OP.mult,
            )
            nc.scalar.activation(sc_t[:, 0, :, :], sc_t[:, 0, :, :], AF.Copy, bias=-1.0)
            # scatter the planes into the interleaved output (2x-mode copies)
            nc.vector.tensor_copy(ov_col[:, :, :, 1], sc_t[:, 1, :, :])
            nc.vector.tensor_copy(ov_col[:, :, :, 0], sc_t[:, 0, :, :])

            # bbox broadcast across classes (free-dim stride-0), per anchor,
            # split DVE / ScalarE by class range
            for a in range(A):
                src = bb[:, :, :, a, :]
                nc.vector.tensor_copy(
                    ov[:, 0:BSPLIT, :, a, 2:6],
                    src.to_broadcast([128, BSPLIT, Q, 4]),
                )
                nc.scalar.copy(
                    ov[:, BSPLIT:C, :, a, 2:6],
                    src.to_broadcast([128, C - BSPLIT, Q, 4]),
                )

            # ---- store: DRAM (p, c, k) = c*115200 + st*23040 + p*180 + k ----
            nc.sync.dma_start(
                out=out_d[:, c0 * ROW : (c0 + ST_CELLS) * ROW].rearrange(
                    "c (p k) -> p c k", p=128
                ),
                in_=ov[:, :, :, :, :].rearrange("p c q a e -> p c (q a e)"),
            )

    nc.finalize()
    return nc


def make_consts(anchor, offset, stride_f):
    """Pack [offs | hanch | cpat] into one (128, F) f32 blob."""
    off = np.asarray(offset, dtype=np.float32).reshape(-1, 2)[:HW_CELLS] * stride_f
    # offs[p, st, q, a, k] = off[st*1280 + p*10 + q, k]
    o = off.reshape(N_ST, 128, Q, 1, 2)                      # (st, p, q, 1, k)
    o = np.broadcast_to(o, (N_ST, 128, Q, NUM_ANCHOR, 2))    # anchors share offs
    offs_cols = np.ascontiguousarray(
        np.transpose(o, (1, 0, 2, 3, 4)).reshape(128, N_ST * Q * NUM_ANCHOR * 2)
    )
    a2 = np.asarray(anchor, dtype=np.float32).reshape(NUM_ANCHOR * 2)
    hanch = np.tile(a2 / 2.0, (128, 1)).astype(np.float32)   # (128, 6)
    cpat = np.tile(np.arange(1, NUM_CLASSES + 1, dtype=np.float32), (128, 1))
    blob = np.concatenate([offs_cols, hanch, cpat], axis=1)
    return np.ascontiguousarray(blob.astype(np.float32))


def _host_prep(output, anchor, offset, stride):
    stride_f = float(stride)
    B = output.shape[0]
    x_all = np.ascontiguousarray(
        np.asarray(output, dtype=np.float32).reshape(B, HW_CELLS, NUM_ANCHOR * NUM_PRED)
    )
    consts = make_consts(anchor, offset, stride_f)
    return stride_f, x_all, consts


def kernel(output, anchor, offset, stride):
    from concourse.bass_utils import run_bass_kernel_spmd

    stride_f, x_all, consts = _host_prep(output, anchor, offset, stride)
    key = ("nc", stride_f)
    if key not in _CACHE:
        _CACHE[key] = _build(stride_f)
    nc = _CACHE[key]

    in_maps = [{"x": x_all[b], "consts": consts} for b in range(N_CORES)]
    res = run_bass_kernel_spmd(
        nc,
        in_maps,
        list(range(N_CORES)),
        tmpdir=os.environ.get("KERNEL_TRACE_DIR") or None,
    )
    global LAST_RESULT
    LAST_RESULT = res
    outs = [
        r["out"].reshape(NUM_CLASSES * HW_CELLS * NUM_ANCHOR, 6) for r in res.results
    ]
    return np.stack(outs, axis=0)


if __name__ == "__main__":
    rng = np.random.default_rng(0)
    out = rng.standard_normal((8, 80, 80, 255), dtype=np.float32)
    anchor = rng.uniform(10.0, 120.0, (1, 1, 3, 2)).astype(np.float32)
    gy, gx = np.meshgrid(np.arange(80, dtype=np.float32), np.arange(80, dtype=np.float32), indexing="ij")
    offset = np.stack([gx, gy], axis=-1).reshape(1, 80, 80, 1, 2)
    r = kernel(out, anchor, offset, 8)
    print(r.shape, r.dtype)


# revision 9
# speedup vs baseline: 1.4220x; 1.1866x over previous
"""Trainium2 Bass kernel for YOLO-style detection decode (nms_detection).

Computes, for input `output` (B=8, H=80, W=80, A*85=255):
  per (b, cell, anchor):  xy = (sigmoid(txy) + grid_off) * stride
                          wh = exp(twh) * anchor
                          bbox = [xy - wh/2, xy + wh/2]
                          p_c = sigmoid(cls_c) * sigmoid(obj)
  out (B, C*hw*A, 6) rows = [cid, score, x1, y1, x2, y2] where
  cid = c if p_c > 0.01 else -1, score = p_c if p_c > 0.01 else 0.

Sharding: pure data parallel over batch, one batch element per NeuronCore.

Per-core design (output is 37 MB/core -> store-bandwidth bound):
  - fully CELL-MAJOR pipeline: partition p owns q consecutive cells of each
    128*q-cell supertile. No transposes, no PSUM, no TensorE at all; every op
    runs on all 128 partitions.
  - output staging tiles are [128, 40, q, A, 6] (class in the FREE dim, two
    40-class halves); the store DMA's DRAM-side AP (p, c, k) =
    c*115200 + c0*18 + p*q*18 + k writes q*72-byte contiguous runs per
    (partition, class) - all 16 SDMA engines carry equal load.
  - supertile schedule [4, 14, 16, 16]*128 cells: the small first tile gets
    the first store in flight early; the big tiles give 1008/1152B DMA
    descriptors (>=512B line-rate).
  - the two class-halves use bufs=1 tiles: store(half, st) overlaps
    assembly of the other half / next supertile.
  - score & cid each use one fused scalar_tensor_tensor:
      score = (S > t) * S;  cid+1 = (S > t) * (c+1), then ScalarE adds -1.
  - bbox columns are broadcast across classes with free-dim stride-0 APs,
    split between DVE (2 elem/cyc copies) and ScalarE.
  - exp(x) = sigmoid(x)/sigmoid(-x) so ScalarE never switches tables.
"""

import sys
import os
from contextlib import ExitStack

if "/opt/trn_rl_repo" not in sys.path:
    sys.path.insert(0, "/opt/trn_rl_repo")

import numpy as np

NUM_CLASSES = 80
NUM_ANCHOR = 3
NUM_PRED = 85
HW_CELLS = 6400
THRESH = 0.01
N_CORES = 8
ROW = 6 * NUM_ANCHOR  # f32 per cell per class in the output (18)

# cells-per-partition for each supertile; sum must be HW_CELLS/128 = 50
QS = tuple(int(x) for x in os.environ.get("KERNEL_QS", "4,14,16,16").split(","))
assert sum(QS) == HW_CELLS // 128

CHALF = NUM_CLASSES // 2  # classes per store half (40)
# within each half, classes [0, BSP) go to DVE, [BSP, CHALF) to ScalarE
BSP = int(os.environ.get("KERNEL_BSP", "22"))

_CACHE = {}
LAST_RESULT = None  # BassKernelResults of the most recent kernel() call


def _build(stride_f: float):
    import concourse.bass as bass  # noqa: F401
    import concourse.bacc as bacc
    import concourse.tile as tile
    from concourse import mybir

    f32 = mybir.dt.float32
    AF = mybir.ActivationFunctionType
    OP = mybir.AluOpType

    C = NUM_CLASSES
    A = NUM_ANCHOR

    # consts blob: [offs (50*A*2) | hanch (A*2) | cpat (C)]
    OFF_HANCH = 50 * A * 2         # 300
    OFF_CPAT = OFF_HANCH + A * 2   # 306
    CONST_F = OFF_CPAT + C         # 386

    nc = bacc.Bacc("TRN2", target_bir_lowering=False, debug=False)
    x_d = nc.declare_dram_parameter("x", [HW_CELLS, A * NUM_PRED], f32, isOutput=False)
    const_d = nc.declare_dram_parameter("consts", [128, CONST_F], f32, isOutput=False)
    out_d = nc.declare_dram_parameter("out", [C, HW_CELLS * ROW], f32, isOutput=True)

    with ExitStack() as ctx:
        tc = ctx.enter_context(tile.TileContext(nc))
        cpool = ctx.enter_context(tc.tile_pool(name="const", bufs=1))
        in_pool = ctx.enter_context(tc.tile_pool(name="inp", bufs=2))
        sm_pool = ctx.enter_context(tc.tile_pool(name="small", bufs=2))
        s_pool = ctx.enter_context(tc.tile_pool(name="scls", bufs=2))
        oa_pool = ctx.enter_context(tc.tile_pool(name="outa", bufs=1))
        ob_pool = ctx.enter_context(tc.tile_pool(name="outb", bufs=1))

        # ---- constants (one DMA -> one sem lane) ----
        const_sb = cpool.tile([128, CONST_F], f32, tag="consts")
        nc.gpsimd.dma_start(out=const_sb[:, :], in_=const_d[:, :])
        hanch_v = const_sb[:, OFF_HANCH:OFF_CPAT].rearrange(
            "p (u a k) -> p u a k", a=A, k=2
        )
        cpat_v = const_sb[:, OFF_CPAT:CONST_F].rearrange("p (c u) -> p c u", u=1)

        # ---- warm-up: let each engine observe the const DMA once, so no
        # later instruction needs more than one sync-wait (ISA limit) ----
        warm = cpool.tile([128, 4], f32, tag="warm")
        nc.vector.tensor_copy(warm[0:1, 0:1], const_sb[0:1, 0:1])
        nc.scalar.copy(warm[0:1, 1:2], const_sb[0:1, 0:1])
        nc.gpsimd.tensor_copy(warm[0:1, 2:3], const_sb[0:1, 0:1])

        qoff = 0
        for st, q in enumerate(QS):
            cells = 128 * q
            c0 = 128 * qoff  # starting cell = partition0's first cell offset... (layout below)

            # ---- load: partition p holds cells [c0 + q*p, c0 + q*p + q) ----
            in_t = in_pool.tile([128, QS[-1], A * NUM_PRED], f32, tag="in")
            nc.gpsimd.dma_start(
                out=in_t[:, 0:q, :],
                in_=x_d[c0 : c0 + cells, :].rearrange("(p q) c -> p q c", p=128),
            )
            in_v = in_t[:, 0:q, :].rearrange("p q (a c) -> p q a c", a=A, c=NUM_PRED)

            # exp(wh) = sigmoid(wh)/sigmoid(-wh); sgnw reads raw wh so it runs
            # before the in-place sigmoid (same engine keeps them ordered)
            sgnw = sm_pool.tile([128, QS[-1], A, 2], f32, tag="sgnw")
            nc.scalar.activation(
                sgnw[:, 0:q, :, :], in_v[:, :, :, 2:4], AF.Sigmoid, scale=-1.0
            )
            # sigmoid of everything, in place
            nc.scalar.activation(in_t[:, 0:q, :], in_t[:, 0:q, :], AF.Sigmoid)
            sig_v = in_v

            rec = sm_pool.tile([128, QS[-1], A, 2], f32, tag="rec")
            nc.vector.reciprocal(rec[:, 0:q, :, :], sgnw[:, 0:q, :, :])
            t1 = sm_pool.tile([128, QS[-1], A, 2], f32, tag="t1")
            nc.vector.tensor_tensor(
                t1[:, 0:q, :, :],
                sig_v[:, :, :, 2:4],
                hanch_v.to_broadcast([128, q, A, 2]),
                OP.mult,
            )
            halfwh = sm_pool.tile([128, QS[-1], A, 2], f32, tag="halfwh")
            nc.vector.tensor_tensor(
                halfwh[:, 0:q, :, :], t1[:, 0:q, :, :], rec[:, 0:q, :, :], OP.mult
            )

            # xy = sigmoid(xy)*stride + off*stride
            xy = sm_pool.tile([128, QS[-1], A, 2], f32, tag="xy")
            nc.vector.scalar_tensor_tensor(
                xy[:, 0:q, :, :],
                in0=sig_v[:, :, :, 0:2],
                scalar=stride_f,
                in1=const_sb[:, qoff * A * 2 : (qoff + q) * A * 2].rearrange(
                    "p (q a k) -> p q a k", a=A, k=2
                ),
                op0=OP.mult,
                op1=OP.add,
            )

            # bbox cell-major: [p, 1, q, a, 0:2]=xy-halfwh, [2:4]=xy+halfwh
            bb = sm_pool.tile([128, 1, QS[-1], A, 4], f32, tag="bb")
            nc.vector.tensor_tensor(
                bb[:, 0, 0:q, :, 0:2], xy[:, 0:q, :, :], halfwh[:, 0:q, :, :], OP.subtract
            )
            nc.vector.tensor_tensor(
                bb[:, 0, 0:q, :, 2:4], xy[:, 0:q, :, :], halfwh[:, 0:q, :, :], OP.add
            )

            # class scores S[p, q, a, c] = sigmoid(cls) * sigmoid(obj)
            S = s_pool.tile([128, QS[-1], A, C], f32, tag="S")
            nc.gpsimd.tensor_tensor(
                S[:, 0:q, :, :],
                sig_v[:, :, :, 5:85],
                sig_v[:, :, :, 4:5].to_broadcast([128, q, A, C]),
                OP.mult,
            )
            S_cqa = S[:, 0:q, :, :].rearrange("p q a c -> p c (q a)")

            # ---- per class-half: assemble + store ----
            for h, pool in ((0, oa_pool), (1, ob_pool)):
                cl = h * CHALF
                ov = pool.tile([128, CHALF, QS[-1], A, 6], f32, tag=f"ov{h}")
                ov_col = ov[:, :, 0:q, :, :].rearrange("p c q a e -> p c (q a) e")

                # score = (S > t) * S
                nc.vector.scalar_tensor_tensor(
                    ov_col[:, :, :, 1],
                    in0=S_cqa[:, cl : cl + CHALF, :],
                    scalar=THRESH,
                    in1=S_cqa[:, cl : cl + CHALF, :],
                    op0=OP.is_gt,
                    op1=OP.mult,
                )
                # cid+1 = (S > t) * (c+1); ScalarE applies the -1
                nc.vector.scalar_tensor_tensor(
                    ov_col[:, :, :, 0],
                    in0=S_cqa[:, cl : cl + CHALF, :],
                    scalar=THRESH,
                    in1=cpat_v[:, cl : cl + CHALF, :].to_broadcast([128, CHALF, q * A]),
                    op0=OP.is_gt,
                    op1=OP.mult,
                )
                nc.scalar.activation(
                    ov_col[:, :, :, 0], ov_col[:, :, :, 0], AF.Copy, bias=-1.0
                )

                # bbox broadcast across classes, per anchor, DVE/ScalarE split
                for a in range(A):
                    src = bb[:, :, 0:q, a, :]
                    nc.vector.tensor_copy(
                        ov[:, 0:BSP, 0:q, a, 2:6],
                        src.to_broadcast([128, BSP, q, 4]),
                    )
                    nc.scalar.copy(
                        ov[:, BSP:CHALF, 0:q, a, 2:6],
                        src.to_broadcast([128, CHALF - BSP, q, 4]),
                    )

                # store: DRAM (p, c, k) = (cl+c)*115200 + c0*18 + p*q*18 + k
                nc.sync.dma_start(
                    out=out_d[
                        cl : cl + CHALF, c0 * ROW : (c0 + cells) * ROW
                    ].rearrange("c (p k) -> p c k", p=128),
                    in_=ov[:, :, 0:q, :, :].rearrange("p c q a e -> p c (q a e)"),
                )
            qoff += q

    nc.finalize()
    return nc


def make_consts(anchor, offset, stride_f):
    """Pack [offs | hanch | cpat] into one (128, F) f32 blob."""
    off = np.asarray(offset, dtype=np.float32).reshape(-1, 2)[:HW_CELLS] * stride_f
    # offs[p, qoff+i, a, k] = off[c0 + p*q + i, k] for supertile at (c0, q)
    cols = []
    c0 = 0
    for q in QS:
        o = off[c0 : c0 + 128 * q].reshape(128, q, 1, 2)
        cols.append(np.broadcast_to(o, (128, q, NUM_ANCHOR, 2)).reshape(128, -1))
        c0 += 128 * q
    offs_cols = np.ascontiguousarray(np.concatenate(cols, axis=1))
    a2 = np.asarray(anchor, dtype=np.float32).reshape(NUM_ANCHOR * 2)
    hanch = np.tile(a2 / 2.0, (128, 1)).astype(np.float32)   # (128, 6)
    cpat = np.tile(np.arange(1, NUM_CLASSES + 1, dtype=np.float32), (128, 1))
    blob = np.concatenate([offs_cols, hanch, cpat], axis=1)
    return np.ascontiguousarray(blob.astype(np.float32))


def _host_prep(output, anchor, offset, stride):
    stride_f = float(stride)
    B = output.shape[0]
    x_all = np.ascontiguousarray(
        np.asarray(output, dtype=np.float32).reshape(B, HW_CELLS, NUM_ANCHOR * NUM_PRED)
    )
    consts = make_consts(anchor, offset, stride_f)
    return stride_f, x_all, consts


def kernel(output, anchor, offset, stride):
    from concourse.bass_utils import run_bass_kernel_spmd

    stride_f, x_all, consts = _host_prep(output, anchor, offset, stride)
    key = ("nc", stride_f)
    if key not in _CACHE:
        _CACHE[key] = _build(stride_f)
    nc = _CACHE[key]

    in_maps = [{"x": x_all[b], "consts": consts} for b in range(N_CORES)]
    res = run_bass_kernel_spmd(
        nc,
        in_maps,
        list(range(N_CORES)),
        tmpdir=os.environ.get("KERNEL_TRACE_DIR") or None,
    )
    global LAST_RESULT
    LAST_RESULT = res
    outs = [
        r["out"].reshape(NUM_CLASSES * HW_CELLS * NUM_ANCHOR, 6) for r in res.results
    ]
    return np.stack(outs, axis=0)


if __name__ == "__main__":
    rng = np.random.default_rng(0)
    out = rng.standard_normal((8, 80, 80, 255), dtype=np.float32)
    anchor = rng.uniform(10.0, 120.0, (1, 1, 3, 2)).astype(np.float32)
    gy, gx = np.meshgrid(np.arange(80, dtype=np.float32), np.arange(80, dtype=np.float32), indexing="ij")
    offset = np.stack([gx, gy], axis=-1).reshape(1, 80, 80, 1, 2)
    r = kernel(out, anchor, offset, 8)
    print(r.shape, r.dtype)
